# revision 1
# baseline (speedup 1.0000x reference)
"""GRU kernel for Trainium2, 8 NeuronCores, data-parallel over batch.

Strategy
--------
reference:  per step t (T=512):
    gi = [h, x_t]; r = sig(gi@Wr+br); z = sig(gi@Wz+bz)
    hh = tanh([h*r, x_t]@Wl+bl); h = (1-z)h + z*hh; out_t = relu(h@Wo+bo)

Decomposition per core (B_local=8 rows):
  Phase 1 (parallel over all t): XgT = Wx_g^T @ x^T + b_g for g in {r,z,l}
     (f32r matmuls, N=512) -> DRAM, transposed layout [H, B_local*T].
  Recurrence (serial, fully transposed domain; state hT [128 part, 8 chunks*8b]):
     per step: ar^T/az^T = Wh^T h^T (bf16 stationary weights resident in SBUF,
     LDW+MM pairs at ~33ns), + X slice, sigmoid; rh^T = r^T*h^T;
     al^T likewise from rh^T; h_new^T elementwise. h history accumulates in
     SBUF as bf16 and every 16 steps the output projection
     outT = relu(Wo^T hT + bo) runs fused (bf16), written blockwise to DRAM.
  Host: pre-transposes x per core, un-permutes outT blocks.
"""
import os
import numpy as np
from contextlib import ExitStack

import concourse.bass as bass
import concourse.tile as tile
from concourse import bacc, mybir
from concourse import bass_utils

B, T_FULL, D, H = 64, 512, 1024, 1024
NCORES = 8
BL = B // NCORES            # 8 batch rows per core
KC = H // 128               # 8 contraction chunks
JC = H // 128               # 8 output chunks
BLK = 16                    # recurrence steps per output-projection block

f32 = mybir.dt.float32
f32r = mybir.dt.float32r
bf16 = mybir.dt.bfloat16
AF = mybir.ActivationFunctionType

_CACHE = {}


def build_program(T):
    cols = BL * T           # columns of the transposed activations
    nblk = T // BLK
    assert T % BLK == 0

    nc = bacc.Bacc("TRN2", target_bir_lowering=False, debug=False, num_devices=1)

    xT = nc.dram_tensor("xT", (H, cols), f32, kind="ExternalInput").ap()
    wx = {g: nc.dram_tensor(f"wx{g}", (D, H), f32, kind="ExternalInput").ap()
          for g in "rzl"}
    wh = {g: nc.dram_tensor(f"wh{g}", (H, H), f32, kind="ExternalInput").ap()
          for g in "rzl"}
    bias = {g: nc.dram_tensor(f"b{g}", (H, 1), f32, kind="ExternalInput").ap()
            for g in "rzl"}
    wo_d = nc.dram_tensor("wo", (H, H), f32, kind="ExternalInput").ap()
    bo_d = nc.dram_tensor("bo", (H, 1), f32, kind="ExternalInput").ap()
    outT = nc.dram_tensor("outT", (128, nblk * JC * BLK * BL), f32,
                          kind="ExternalOutput").ap()

    with tile.TileContext(nc) as tc, ExitStack() as top:
        dram = top.enter_context(tc.tile_pool(name="dram", bufs=1, space="DRAM"))
        xg_d = {g: dram.tile([H, cols], f32, tag=f"X{g}", name=f"X{g}") for g in "rzl"}

        # ---------------- Phase 1: x projections (f32r) ----------------
        with ExitStack() as ctx:
            wp = ctx.enter_context(tc.tile_pool(name="p1w", bufs=1))
            xp = ctx.enter_context(tc.tile_pool(name="p1x", bufs=2))
            pp = ctx.enter_context(tc.tile_pool(name="p1ps", bufs=4, space="PSUM"))
            op = ctx.enter_context(tc.tile_pool(name="p1o", bufs=3))
            bp = ctx.enter_context(tc.tile_pool(name="p1b", bufs=1))

            wx_sb = {}
            bt = {}
            for g in "rzl":
                wx_sb[g] = wp.tile([128, KC * H], f32r, tag=f"wx{g}", name=f"wx{g}sb")
                for kc in range(KC):
                    nc.sync.dma_start(
                        wx_sb[g][:, kc * H:(kc + 1) * H],
                        wx[g][kc * 128:(kc + 1) * 128, :].bitcast(f32r))
                bt[g] = bp.tile([128, JC], f32, tag=f"b{g}", name=f"bt{g}")
                for jc in range(JC):
                    nc.sync.dma_start(bt[g][:, jc:jc + 1],
                                      bias[g][jc * 128:(jc + 1) * 128, :])

            NCB = 512
            for cb in range(cols // NCB):
                xt = xp.tile([128, KC * NCB], f32r, tag="xt")
                for kc in range(KC):
                    nc.sync.dma_start(
                        xt[:, kc * NCB:(kc + 1) * NCB],
                        xT[kc * 128:(kc + 1) * 128,
                           cb * NCB:(cb + 1) * NCB].bitcast(f32r))
                for g in "rzl":
                    for jc in range(JC):
                        ps = pp.tile([128, NCB], f32, tag="ps")
                        for kc in range(KC):
                            nc.tensor.matmul(
                                ps[:],
                                lhsT=wx_sb[g][:, kc * H + jc * 128:
                                              kc * H + (jc + 1) * 128],
                                rhs=xt[:, kc * NCB:(kc + 1) * NCB],
                                start=(kc == 0), stop=(kc == KC - 1))
                        ot = op.tile([128, NCB], f32, tag="ot")
                        nc.scalar.activation(ot[:], ps[:], AF.Identity,
                                             bias=bt[g][:, jc:jc + 1])
                        nc.sync.dma_start(
                            xg_d[g][jc * 128:(jc + 1) * 128,
                                    cb * NCB:(cb + 1) * NCB], ot[:])

        # Phase-1 writes X* to DRAM via DMA; DRAM-tile RAW deps are not
        # reliably tracked by the scheduler, so fence before consuming.
        tc.strict_bb_all_engine_barrier()

        # ------------- Recurrence + fused output projection -------------
        with ExitStack() as ctx:
            wp = ctx.enter_context(tc.tile_pool(name="rw", bufs=1))
            sg = ctx.enter_context(tc.tile_pool(name="stg", bufs=2))
            xb = ctx.enter_context(tc.tile_pool(name="xblk", bufs=2))
            hi = ctx.enter_context(tc.tile_pool(name="hist", bufs=2))
            st = ctx.enter_context(tc.tile_pool(name="state", bufs=2))
            el = ctx.enter_context(tc.tile_pool(name="elt", bufs=2))
            pg = ctx.enter_context(tc.tile_pool(name="psg", bufs=2, space="PSUM"))
            p3 = ctx.enter_context(tc.tile_pool(name="ps3", bufs=2, space="PSUM"))
            o3 = ctx.enter_context(tc.tile_pool(name="o3", bufs=3))
            bp = ctx.enter_context(tc.tile_pool(name="rb", bufs=1))

            # resident bf16 weights (staged through f32)
            wh_sb = {}
            for g in "rzl":
                wh_sb[g] = wp.tile([128, KC * H], bf16, tag=f"wh{g}", name=f"wh{g}sb")
                for kc in range(KC):
                    stg = sg.tile([128, H], f32, tag="stg")
                    nc.sync.dma_start(stg[:], wh[g][kc * 128:(kc + 1) * 128, :])
                    nc.vector.tensor_copy(wh_sb[g][:, kc * H:(kc + 1) * H], stg[:])
            wo_sb = wp.tile([128, KC * H], bf16, tag="wo")
            for kc in range(KC):
                stg = sg.tile([128, H], f32, tag="stg")
                nc.sync.dma_start(stg[:], wo_d[kc * 128:(kc + 1) * 128, :])
                nc.vector.tensor_copy(wo_sb[:, kc * H:(kc + 1) * H], stg[:])
            bo_t = bp.tile([128, JC], f32, tag="bo")
            for jc in range(JC):
                nc.sync.dma_start(bo_t[:, jc:jc + 1],
                                  bo_d[jc * 128:(jc + 1) * 128, :])

            CW = BL * KC        # 64: columns of a state tile (chunk-major, b minor)
            hT = st.tile([128, CW], f32, tag="hT")
            nc.vector.memset(hT[:], 0.0)
            hz = bp.tile([128, CW], bf16, tag="h0")
            nc.vector.memset(hz[:], 0.0)
            hprev_src, hprev_off = hz, 0       # bf16 h^T of previous step

            def gate_mm(ps, wt, src, off):
                for jc in range(JC):
                    for kc in range(KC):
                        nc.tensor.matmul(
                            ps[:, jc * BL:(jc + 1) * BL],
                            lhsT=wt[:, (kc * JC + jc) * 128:
                                    (kc * JC + jc + 1) * 128],
                            rhs=src[:, off + kc * BL:off + (kc + 1) * BL],
                            start=(kc == 0), stop=(kc == KC - 1))

            for bi in range(nblk):
                xblk = {}
                for g in "rzl":
                    xblk[g] = xb.tile([128, KC * BLK * BL], f32, tag=f"xb{g}", name=f"xb{g}t")
                    for kc in range(KC):
                        nc.sync.dma_start(
                            xblk[g][:, kc * BLK * BL:(kc + 1) * BLK * BL],
                            xg_d[g][kc * 128:(kc + 1) * 128,
                                    bi * BLK * BL:(bi + 1) * BLK * BL])
                hist = hi.tile([128, BLK * CW], bf16, tag="hist")

                for dt in range(BLK):
                    def xsl(g):
                        return (xblk[g][:].rearrange("p (c s) -> p c s", c=KC)
                                [:, :, dt * BL:(dt + 1) * BL])
                    psr = pg.tile([128, CW], f32, tag="gr")
                    gate_mm(psr, wh_sb["r"], hprev_src, hprev_off)
                    psz = pg.tile([128, CW], f32, tag="gz")
                    gate_mm(psz, wh_sb["z"], hprev_src, hprev_off)

                    c3 = "p (c b) -> p c b"
                    tr = el.tile([128, CW], f32, tag="tr")
                    nc.vector.tensor_add(tr[:].rearrange(c3, c=KC),
                                         psr[:].rearrange(c3, c=KC), xsl("r"))
                    r = el.tile([128, CW], f32, tag="r")
                    nc.scalar.activation(r[:], tr[:], AF.Sigmoid)
                    rh = el.tile([128, CW], bf16, tag="rh")
                    nc.vector.tensor_mul(rh[:], r[:], hT[:])

                    psl = pg.tile([128, CW], f32, tag="gl")
                    gate_mm(psl, wh_sb["l"], rh, 0)

                    tz = el.tile([128, CW], f32, tag="tz")
                    nc.vector.tensor_add(tz[:].rearrange(c3, c=KC),
                                         psz[:].rearrange(c3, c=KC), xsl("z"))
                    z = el.tile([128, CW], f32, tag="z")
                    nc.scalar.activation(z[:], tz[:], AF.Sigmoid)

                    tl = el.tile([128, CW], f32, tag="tl")
                    nc.vector.tensor_add(tl[:].rearrange(c3, c=KC),
                                         psl[:].rearrange(c3, c=KC), xsl("l"))
                    hh = el.tile([128, CW], f32, tag="hh")
                    nc.scalar.activation(hh[:], tl[:], AF.Tanh)

                    d = el.tile([128, CW], f32, tag="d")
                    nc.vector.tensor_sub(d[:], hh[:], hT[:])
                    e = el.tile([128, CW], f32, tag="e")
                    nc.vector.tensor_mul(e[:], z[:], d[:])
                    hTn = st.tile([128, CW], f32, tag="hT")
                    nc.vector.tensor_add(hTn[:], hT[:], e[:])
                    nc.vector.tensor_copy(hist[:, dt * CW:(dt + 1) * CW], hTn[:])
                    hT = hTn
                    hprev_src, hprev_off = hist, dt * CW

                # fused output projection for this block (bf16).
                # Compact the strided (t, c, b) history view into contiguous
                # per-k-chunk rhs tiles first.
                hv = hist[:].rearrange("p (t c b) -> p t c b", t=BLK, c=KC)
                hcmp = o3.tile([128, KC * BLK * BL], bf16, tag="hcmp",
                               name="hcmp")
                for kc in range(KC):
                    nc.vector.tensor_copy(
                        hcmp[:, kc * BLK * BL:(kc + 1) * BLK * BL]
                        .rearrange("p (t b) -> p t b", t=BLK),
                        hv[:, :, kc, :])
                for jc in range(JC):
                    pso = p3.tile([128, BLK * BL], f32, tag="pso")
                    for kc in range(KC):
                        nc.tensor.matmul(
                            pso[:],
                            lhsT=wo_sb[:, (kc * JC + jc) * 128:
                                       (kc * JC + jc + 1) * 128],
                            rhs=hcmp[:, kc * BLK * BL:(kc + 1) * BLK * BL],
                            start=(kc == 0), stop=(kc == KC - 1))
                    ou = o3.tile([128, BLK * BL], f32, tag="ou")
                    nc.scalar.activation(ou[:], pso[:], AF.Relu,
                                         bias=bo_t[:, jc:jc + 1])
                    nc.sync.dma_start(
                        outT[:, (bi * JC + jc) * BLK * BL:
                             (bi * JC + jc + 1) * BLK * BL], ou[:])

    nc.compile()
    return nc


def get_program(T):
    if T not in _CACHE:
        _CACHE[T] = build_program(T)
    return _CACHE[T]


def kernel(input, Wr, br, Wz, bz, Wl, bl, Wo, bo):
    Tt = input.shape[1]
    prog = get_program(Tt)
    cols = BL * Tt

    w_common = {
        "wxr": np.ascontiguousarray(Wr[H:]), "whr": np.ascontiguousarray(Wr[:H]),
        "wxz": np.ascontiguousarray(Wz[H:]), "whz": np.ascontiguousarray(Wz[:H]),
        "wxl": np.ascontiguousarray(Wl[H:]), "whl": np.ascontiguousarray(Wl[:H]),
        "br": np.ascontiguousarray(br.reshape(H, 1)),
        "bz": np.ascontiguousarray(bz.reshape(H, 1)),
        "bl": np.ascontiguousarray(bl.reshape(H, 1)),
        "wo": np.ascontiguousarray(Wo),
        "bo": np.ascontiguousarray(bo.reshape(H, 1)),
    }
    in_maps = []
    for c in range(NCORES):
        xl = np.asarray(input[c * BL:(c + 1) * BL], dtype=np.float32)
        xTl = np.ascontiguousarray(xl.transpose(2, 1, 0).reshape(H, cols))
        in_maps.append({"xT": xTl, **w_common})

    res = bass_utils.run_bass_kernel_spmd(prog, in_maps,
                                          core_ids=list(range(NCORES)))
    nblk = Tt // BLK
    outs = []
    for c in range(NCORES):
        oT = res.results[c]["outT"]              # [128, nblk*JC*BLK*BL]
        o = oT.reshape(128, nblk, JC, BLK, BL)   # p, bi, j, dt, b
        o = o.transpose(4, 1, 3, 2, 0).reshape(BL, Tt, H)
        outs.append(o)
    return np.ascontiguousarray(np.concatenate(outs, axis=0))



# revision 6
# speedup vs baseline: 1.0537x; 1.0537x over previous
"""GRU kernel for Trainium2, 8 NeuronCores, data-parallel over batch.

Strategy (v2)
-------------
reference:  per step t (T=512):
    gi = [h, x_t]; r = sig(gi@Wr+br); z = sig(gi@Wz+bz)
    hh = tanh([h*r, x_t]@Wl+bl); h = (1-z)h + z*hh; out_t = relu(h@Wo+bo)

Decomposition per core (B_local=8 rows, fully transposed domain;
state h^T lives as bf16 [128 part, kc-major 8 chunks x 8 batch cols]):

  Phase 1 (parallel over t): X_g^T = Wx_g^T x^T + b_g, g in {r,z,l}
    (f32r matmuls, N=512) -> DRAM as bf16, layout [H, B_local*T].

  Recurrence (serial over t). Per step:
    - r gate: per out-chunk jc, PSUM initialized with X_r via an
      identity-stationary matmul, then 8 kc matmuls of Wh_r^T h^T
      (bf16 resident weights).  sigmoid straight out of PSUM.
    - z gate: plain matmul accumulation + DVE add of X_z + sigmoid
      (its serial chain hides under the l-gate matmuls).
    - l gate: like r (identity-fold of X_l), rhs = (r*h)^T.
    - update: w = (1-z)*h precomputed during the l window;
      h_new = w + z*tanh(psl)  written directly as bf16 into the
      chunk-major history tile (no separate f32 state, no cast op).
    - every 2nd step, one jc-chunk of the previous block's output
      projection relu(Wo^T h^T + bo) is issued to fill the PE idle
      tail (keeps pairs flowing between the l gate and next r gate).

  PSUM gate pool is 4 deep so step t's gates never wait on step t-1's
  tail reads (the v1 bufs=2 pool serialized exactly that way).
"""
import numpy as np
from contextlib import ExitStack

import concourse.bass as bass
import concourse.tile as tile
from concourse import bacc, mybir
from concourse import bass_utils

B, T_FULL, D, H = 64, 512, 1024, 1024
NCORES = 8
BL = B // NCORES            # 8 batch rows per core
KC = H // 128               # 8 contraction chunks
JC = H // 128               # 8 output chunks
BLK = 16                    # recurrence steps per output-projection block

f32 = mybir.dt.float32
f32r = mybir.dt.float32r
bf16 = mybir.dt.bfloat16
AF = mybir.ActivationFunctionType
ALU = mybir.AluOpType

_CACHE = {}


def build_program(T):
    cols = BL * T           # columns of the transposed activations
    nblk = T // BLK
    assert T % BLK == 0
    CW = BL * KC            # 64: cols of a state tile (kc-major, b minor)

    nc = bacc.Bacc("TRN2", target_bir_lowering=False, debug=False, num_devices=1)

    xT = nc.dram_tensor("xT", (H, cols), f32, kind="ExternalInput").ap()
    wx = {g: nc.dram_tensor(f"wx{g}", (D, H), f32, kind="ExternalInput").ap()
          for g in "rzl"}
    wh = {g: nc.dram_tensor(f"wh{g}", (H, H), f32, kind="ExternalInput").ap()
          for g in "rzl"}
    bias = {g: nc.dram_tensor(f"b{g}", (H, 1), f32, kind="ExternalInput").ap()
            for g in "rzl"}
    wo_d = nc.dram_tensor("wo", (H, H), f32, kind="ExternalInput").ap()
    bo_d = nc.dram_tensor("bo", (H, 1), f32, kind="ExternalInput").ap()
    id_d = nc.dram_tensor("ident", (128, 128), f32, kind="ExternalInput").ap()
    outT = nc.dram_tensor("outT", (128, nblk * JC * BLK * BL), f32,
                          kind="ExternalOutput").ap()

    with tile.TileContext(nc) as tc, ExitStack() as top:
        dram = top.enter_context(tc.tile_pool(name="dram", bufs=1, space="DRAM"))
        xg_d = {g: dram.tile([H, cols], bf16, tag=f"X{g}", name=f"X{g}")
                for g in "rzl"}

        # ---------------- Phase 1: x projections (f32r) ----------------
        with ExitStack() as ctx:
            wp = ctx.enter_context(tc.tile_pool(name="p1w", bufs=1))
            xp = ctx.enter_context(tc.tile_pool(name="p1x", bufs=2))
            pp = ctx.enter_context(tc.tile_pool(name="p1ps", bufs=4, space="PSUM"))
            op = ctx.enter_context(tc.tile_pool(name="p1o", bufs=3))
            bp = ctx.enter_context(tc.tile_pool(name="p1b", bufs=1))

            wx_sb = {}
            bt = {}
            for g in "rzl":
                wx_sb[g] = wp.tile([128, KC * H], f32r, tag=f"wx{g}", name=f"wx{g}sb")
                for kc in range(KC):
                    nc.sync.dma_start(
                        wx_sb[g][:, kc * H:(kc + 1) * H],
                        wx[g][kc * 128:(kc + 1) * 128, :].bitcast(f32r))
                bt[g] = bp.tile([128, JC], f32, tag=f"b{g}", name=f"bt{g}")
                for jc in range(JC):
                    nc.sync.dma_start(bt[g][:, jc:jc + 1],
                                      bias[g][jc * 128:(jc + 1) * 128, :])

            NCB = 512
            for cb in range(cols // NCB):
                xt = xp.tile([128, KC * NCB], f32r, tag="xt")
                for kc in range(KC):
                    nc.sync.dma_start(
                        xt[:, kc * NCB:(kc + 1) * NCB],
                        xT[kc * 128:(kc + 1) * 128,
                           cb * NCB:(cb + 1) * NCB].bitcast(f32r))
                for g in "rzl":
                    for jc in range(JC):
                        ps = pp.tile([128, NCB], f32, tag="ps")
                        for kc in range(KC):
                            nc.tensor.matmul(
                                ps[:],
                                lhsT=wx_sb[g][:, kc * H + jc * 128:
                                              kc * H + (jc + 1) * 128],
                                rhs=xt[:, kc * NCB:(kc + 1) * NCB],
                                start=(kc == 0), stop=(kc == KC - 1))
                        ot = op.tile([128, NCB], bf16, tag="ot")
                        nc.scalar.activation(ot[:], ps[:], AF.Identity,
                                             bias=bt[g][:, jc:jc + 1])
                        nc.sync.dma_start(
                            xg_d[g][jc * 128:(jc + 1) * 128,
                                    cb * NCB:(cb + 1) * NCB], ot[:])

        # Phase-1 writes X* to DRAM via DMA; DRAM-tile RAW deps are not
        # reliably tracked by the scheduler, so fence before consuming.
        tc.strict_bb_all_engine_barrier()

        # ------------- Recurrence + fused output projection -------------
        with ExitStack() as ctx:
            wp = ctx.enter_context(tc.tile_pool(name="rw", bufs=1))
            sg = ctx.enter_context(tc.tile_pool(name="stg", bufs=2))
            xb = ctx.enter_context(tc.tile_pool(name="xblk", bufs=2))
            hi = ctx.enter_context(tc.tile_pool(name="hist", bufs=2))
            el = ctx.enter_context(tc.tile_pool(name="elt", bufs=2))
            pgr = ctx.enter_context(tc.tile_pool(name="psr", bufs=2, space="PSUM"))
            pgz = ctx.enter_context(tc.tile_pool(name="psz", bufs=2, space="PSUM"))
            pgl = ctx.enter_context(tc.tile_pool(name="psl", bufs=2, space="PSUM"))
            p3 = ctx.enter_context(tc.tile_pool(name="ps3", bufs=2, space="PSUM"))
            o3 = ctx.enter_context(tc.tile_pool(name="o3", bufs=3))
            bp = ctx.enter_context(tc.tile_pool(name="rb", bufs=1))

            # resident bf16 weights (staged through f32)
            wh_sb = {}
            for g in "rzl":
                wh_sb[g] = wp.tile([128, KC * H], bf16, tag=f"wh{g}", name=f"wh{g}sb")
                for kc in range(KC):
                    stg = sg.tile([128, H], f32, tag="stg")
                    nc.sync.dma_start(stg[:], wh[g][kc * 128:(kc + 1) * 128, :])
                    nc.vector.tensor_copy(wh_sb[g][:, kc * H:(kc + 1) * H], stg[:])
            wo_sb = wp.tile([128, KC * H], bf16, tag="wo")
            for kc in range(KC):
                stg = sg.tile([128, H], f32, tag="stg")
                nc.sync.dma_start(stg[:], wo_d[kc * 128:(kc + 1) * 128, :])
                nc.vector.tensor_copy(wo_sb[:, kc * H:(kc + 1) * H], stg[:])
            ident = wp.tile([128, 128], bf16, tag="ident")
            stg = sg.tile([128, 128], f32, tag="stg")
            nc.sync.dma_start(stg[:], id_d[:])
            nc.vector.tensor_copy(ident[:], stg[:])
            bo_t = bp.tile([128, JC], f32, tag="bo")
            for jc in range(JC):
                nc.sync.dma_start(bo_t[:, jc:jc + 1],
                                  bo_d[jc * 128:(jc + 1) * 128, :])

            # zero initial state (kc-major layout, one step worth)
            hz = bp.tile([128, CW], bf16, tag="h0")
            nc.vector.memset(hz[:], 0.0)

            hist_prev = None       # previous block's history tile

            def gate_mm(ps, wt, src_slices, xfold=None):
                """Accumulate one gate into ps[:, jc*BL...] for all jc.

                src_slices: per-kc list of [128, BL] bf16 APs (h^T chunks).
                xfold: per-jc [128, BL] bf16 APs added via identity matmul.
                """
                for jc in range(JC):
                    reg = ps[:, jc * BL:(jc + 1) * BL]
                    if xfold is not None:
                        nc.tensor.matmul(reg, lhsT=ident[:],
                                         rhs=xfold[jc], start=True, stop=False)
                    for kc in range(KC):
                        nc.tensor.matmul(
                            reg,
                            lhsT=wt[:, (kc * JC + jc) * 128:
                                    (kc * JC + jc + 1) * 128],
                            rhs=src_slices[kc],
                            start=(xfold is None and kc == 0),
                            stop=(kc == KC - 1))

            def wo_unit(hsrc, bi_out, jc):
                """One jc chunk of the output projection for block bi_out."""
                pso = p3.tile([128, BLK * BL], f32, tag="pso")
                for kc in range(KC):
                    nc.tensor.matmul(
                        pso[:],
                        lhsT=wo_sb[:, (kc * JC + jc) * 128:
                                   (kc * JC + jc + 1) * 128],
                        rhs=hsrc[:, kc * BLK * BL:(kc + 1) * BLK * BL],
                        start=(kc == 0), stop=(kc == KC - 1))
                ou = o3.tile([128, BLK * BL], f32, tag="ou")
                nc.scalar.activation(ou[:], pso[:], AF.Relu,
                                     bias=bo_t[:, jc:jc + 1])
                nc.sync.dma_start(
                    outT[:, (bi_out * JC + jc) * BLK * BL:
                         (bi_out * JC + jc + 1) * BLK * BL], ou[:])

            for bi in range(nblk):
                xblk = {}
                for g in "rzl":
                    xblk[g] = xb.tile([128, JC * BLK * BL], bf16, tag=f"xb{g}",
                                      name=f"xb{g}t")
                    for jc in range(JC):
                        nc.sync.dma_start(
                            xblk[g][:, jc * BLK * BL:(jc + 1) * BLK * BL],
                            xg_d[g][jc * 128:(jc + 1) * 128,
                                    bi * BLK * BL:(bi + 1) * BLK * BL])
                # chunk-major history: col = kc*BLK*BL + t*BL + b
                hist = hi.tile([128, KC * BLK * BL], bf16, tag="hist")
                hview = hist[:].rearrange("p (c t b) -> p c t b", c=KC, t=BLK)

                for dt in range(BLK):
                    if bi == 0 and dt == 0:
                        hsl = [hz[:, kc * BL:(kc + 1) * BL] for kc in range(KC)]
                        hap = hz[:].rearrange("p (c b) -> p c b", c=KC)
                    elif dt == 0:
                        hsl = [hist_prev[:, kc * BLK * BL + (BLK - 1) * BL:
                                         kc * BLK * BL + BLK * BL]
                               for kc in range(KC)]
                        hap = (hist_prev[:]
                               .rearrange("p (c t b) -> p c t b", c=KC, t=BLK)
                               [:, :, BLK - 1, :])
                    else:
                        hsl = [hist[:, kc * BLK * BL + (dt - 1) * BL:
                                    kc * BLK * BL + dt * BL]
                               for kc in range(KC)]
                        hap = hview[:, :, dt - 1, :]

                    def xf(g):
                        return [xblk[g][:, jc * BLK * BL + dt * BL:
                                        jc * BLK * BL + (dt + 1) * BL]
                                for jc in range(JC)]

                    def xap(g):
                        return (xblk[g][:]
                                .rearrange("p (c t b) -> p c t b", c=JC, t=BLK)
                                [:, :, dt, :])

                    c3 = "p (c b) -> p c b"

                    # r gate: X_r folded into PSUM; sigmoid from PSUM
                    psr = pgr.tile([128, CW], f32, tag="gr")
                    gate_mm(psr, wh_sb["r"], hsl, xfold=xf("r"))
                    # z gate: plain accumulation; X_z added on DVE later
                    psz = pgz.tile([128, CW], f32, tag="gz")
                    gate_mm(psz, wh_sb["z"], hsl)

                    r = el.tile([128, CW], f32, tag="r")
                    nc.scalar.activation(r[:], psr[:], AF.Sigmoid)
                    rh = el.tile([128, CW], bf16, tag="rh")
                    nc.vector.tensor_mul(rh[:].rearrange(c3, c=KC),
                                         r[:].rearrange(c3, c=KC), hap)

                    # l gate: X_l folded into PSUM, rhs = (r*h)^T
                    psl = pgl.tile([128, CW], f32, tag="gl")
                    gate_mm(psl, wh_sb["l"],
                            [rh[:, kc * BL:(kc + 1) * BL] for kc in range(KC)],
                            xfold=xf("l"))

                    # z post-chain + w = (1-z)*h  (hides under l matmuls)
                    tz = el.tile([128, CW], f32, tag="tz")
                    nc.vector.tensor_add(tz[:].rearrange(c3, c=KC),
                                         psz[:].rearrange(c3, c=KC), xap("z"))
                    z = el.tile([128, CW], f32, tag="z")
                    nc.scalar.activation(z[:], tz[:], AF.Sigmoid)
                    zm1 = el.tile([128, CW], f32, tag="zm1")
                    nc.vector.tensor_scalar(zm1[:], z[:], -1.0, 1.0,
                                            ALU.mult, ALU.add)
                    w = el.tile([128, CW], f32, tag="w")
                    nc.vector.tensor_mul(w[:].rearrange(c3, c=KC),
                                         zm1[:].rearrange(c3, c=KC), hap)

                    # fill PE idle tail with previous block's Wo chunks
                    if bi > 0 and dt % 2 == 0:
                        wo_unit(hist_prev, bi - 1, dt // 2)

                    # tail: hh = tanh(psl); h_new = w + z*hh -> hist (bf16)
                    hh = el.tile([128, CW], f32, tag="hh")
                    nc.scalar.activation(hh[:], psl[:], AF.Tanh)
                    n = el.tile([128, CW], f32, tag="n")
                    nc.vector.tensor_mul(n[:], z[:], hh[:])
                    nc.vector.tensor_add(hview[:, :, dt, :],
                                         w[:].rearrange(c3, c=KC),
                                         n[:].rearrange(c3, c=KC))

                hist_prev = hist

            # output projection for the final block
            for jc in range(JC):
                wo_unit(hist_prev, nblk - 1, jc)

    nc.compile()
    return nc


def get_program(T):
    if T not in _CACHE:
        _CACHE[T] = build_program(T)
    return _CACHE[T]


def make_inmaps(input, Wr, br, Wz, bz, Wl, bl, Wo, bo):
    Tt = input.shape[1]
    cols = BL * Tt
    w_common = {
        "wxr": np.ascontiguousarray(Wr[H:]), "whr": np.ascontiguousarray(Wr[:H]),
        "wxz": np.ascontiguousarray(Wz[H:]), "whz": np.ascontiguousarray(Wz[:H]),
        "wxl": np.ascontiguousarray(Wl[H:]), "whl": np.ascontiguousarray(Wl[:H]),
        "br": np.ascontiguousarray(br.reshape(H, 1)),
        "bz": np.ascontiguousarray(bz.reshape(H, 1)),
        "bl": np.ascontiguousarray(bl.reshape(H, 1)),
        "wo": np.ascontiguousarray(Wo),
        "bo": np.ascontiguousarray(bo.reshape(H, 1)),
        "ident": np.eye(128, dtype=np.float32),
    }
    in_maps = []
    for c in range(NCORES):
        xl = np.asarray(input[c * BL:(c + 1) * BL], dtype=np.float32)
        xTl = np.ascontiguousarray(xl.transpose(2, 1, 0).reshape(H, cols))
        in_maps.append({"xT": xTl, **w_common})
    return in_maps


def kernel(input, Wr, br, Wz, bz, Wl, bl, Wo, bo):
    Tt = input.shape[1]
    prog = get_program(Tt)
    in_maps = make_inmaps(input, Wr, br, Wz, bz, Wl, bl, Wo, bo)
    res = bass_utils.run_bass_kernel_spmd(prog, in_maps,
                                          core_ids=list(range(NCORES)))
    nblk = Tt // BLK
    outs = []
    for c in range(NCORES):
        oT = res.results[c]["outT"]              # [128, nblk*JC*BLK*BL]
        o = oT.reshape(128, nblk, JC, BLK, BL)   # p, bi, j, dt, b
        o = o.transpose(4, 1, 3, 2, 0).reshape(BL, Tt, H)
        outs.append(o)
    return np.ascontiguousarray(np.concatenate(outs, axis=0))
